# revision 4
# baseline (speedup 1.0000x reference)
"""Trainium2 Bass kernel for nn_CausalBankModel (V=32000, E=256, M=256, T=1024,
B=2, H=1024, W=8) on 8 NeuronCores.

Sharding: rows (b*T + t) are sharded 8 x 256 across cores; each core computes
its 256 rows against the FULL vocab for both readout paths, so the logit
statistics (sum exp, sum l*exp l, sum l, sum l^2, max) and the gate are
entirely core-local — the kernel contains NO collectives, so per-core
execution time is independent of cross-core launch skew.

Host prep: the embedding gather x = emb[chars] and the exact (float64)
decaying-state recurrence run on the host; each core receives its transposed
x window [E, 7+256] and states [M, 256] plus full W1/W2. W2 is repacked into
8 vocab groups of 4000 columns padded to 4096, with pad column 4000 holding
the group row-sum so sum_v l falls out of the matmul for free.

Device layout: rows on partitions (2 blocks of 128) for the W2 matmul and
stats; hidden features on partitions for the W1 matmul. W2 (2 x 65.5 MB bf16)
is streamed from HBM on the sync HWDGE queue in [128, 2048] tiles double
buffered against the TensorE accumulation; logits are staged to DRAM in bf16
on the scalar HWDGE queue between the stats pass and the gated mix.
"""

import numpy as np

import sys

sys.path.insert(0, "/opt/trn_rl_repo")

import ml_dtypes  # noqa: E402

from concourse import bacc, mybir, tile  # noqa: E402
from concourse.bass_utils import run_bass_kernel_spmd  # noqa: E402

F32 = mybir.dt.float32
BF16 = mybir.dt.bfloat16
AF = mybir.ActivationFunctionType
ALU = mybir.AluOpType
X_AXIS = mybir.AxisListType.X

V, E, M, T, B, H, W = 32000, 256, 256, 1024, 2, 1024, 8
N_CORES = 8
CORE_IDS = list(range(N_CORES))
RPC = (B * T) // N_CORES   # 256 rows per core
NRB = RPC // 128           # 2 row blocks per core
HBLK = H // 128            # 8 hidden blocks
NG = 8                     # vocab groups
GW = 4096                  # padded group width
GV = V // NG               # 4000 real vocab cols per group
XW = 264                   # x window cols: 7 history + 256 + 1 pad


def _bf(a):
    return np.ascontiguousarray(np.asarray(a).astype(ml_dtypes.bfloat16))


def build_program(gate_w, gate_b, use_b2):
    """Per-core Bass program; identical on all cores (SPMD), all per-core
    variation arrives via ExternalInputs. gate weights are baked in."""
    nc = bacc.Bacc(None, target_bir_lowering=False)

    xt_d = nc.dram_tensor("xt", [2, 128, XW], BF16, kind="ExternalInput")
    st_d = nc.dram_tensor("st", [2, 128, RPC], BF16, kind="ExternalInput")
    w1l_d = nc.dram_tensor("w1l", [4, 128, H], BF16, kind="ExternalInput")
    w1o_d = nc.dram_tensor("w1o", [16, 128, H], BF16, kind="ExternalInput")
    b1l_d = nc.dram_tensor("b1l", [HBLK, 128, 1], F32, kind="ExternalInput")
    b1o_d = nc.dram_tensor("b1o", [HBLK, 128, 1], F32, kind="ExternalInput")
    w2l_d = nc.dram_tensor("w2l", [NG, HBLK, 128, GW], BF16, kind="ExternalInput")
    w2o_d = nc.dram_tensor("w2o", [NG, HBLK, 128, GW], BF16, kind="ExternalInput")
    b2_d = nc.dram_tensor("b2", [2, NG, 1, GW], BF16, kind="ExternalInput")
    out_d = nc.dram_tensor("out", [NRB, 128, V], BF16, kind="ExternalOutput")

    with tile.TileContext(nc) as tc, tile.ExitStack() as top:
        sb = top.enter_context(tc.tile_pool(name="sb", bufs=1))
        dr = top.enter_context(tc.tile_pool(name="dr", bufs=1, space="DRAM"))
        ld = dr.tile([2, NRB, NG, 128, GW], BF16, name="ld")

        xt = [sb.tile([128, XW], BF16, tag=f"xt{e}", name=f"xt{e}") for e in range(2)]
        st = [sb.tile([128, RPC], BF16, tag=f"st{m}", name=f"st{m}") for m in range(2)]
        for e in range(2):
            nc.sync.dma_start(out=xt[e][:], in_=xt_d[e])
        for m in range(2):
            nc.sync.dma_start(out=st[m][:], in_=st_d[m])
        ones_s = sb.tile([1, 128], BF16, tag="ones1")
        if use_b2:
            nc.vector.memset(ones_s[:], 1.0)

        # per-(path, rb) stat accumulators over the NG vocab groups
        acc = {}
        for p in range(2):
            for rb in range(NRB):
                for nm in ("se", "sa", "sl", "sq", "mx"):
                    acc[(p, rb, nm)] = sb.tile(
                        [128, NG], F32, tag=f"ac{p}{rb}{nm}", name=f"ac{p}{rb}{nm}"
                    )

        ht = [
            sb.tile([128, HBLK, RPC], BF16, tag=f"ht{p}", name=f"ht{p}")
            for p in range(2)
        ]

        # ---------- hidden layers (both paths) ----------
        with (
            tc.tile_pool(name="w1p", bufs=1) as w1p,
            tc.tile_pool(name="ph", bufs=4, space="PSUM") as ph,
        ):
            for path in range(2):
                w1dd, b1dd, nk1 = (
                    (w1l_d, b1l_d, 4) if path == 0 else (w1o_d, b1o_d, 16)
                )
                w1_s = w1p.tile([128, nk1, H], BF16, tag=f"w1{path}", name=f"w1{path}")
                for kk in range(nk1):
                    nc.sync.dma_start(out=w1_s[:, kk, :], in_=w1dd[kk])
                b1_s = w1p.tile([128, HBLK], F32, tag=f"b1{path}", name=f"b1{path}")
                for hh in range(HBLK):
                    nc.sync.dma_start(out=b1_s[:, hh : hh + 1], in_=b1dd[hh])

                def rhs_for(kk):
                    if path == 0:
                        if kk < 2:
                            return st[kk][:, 0:RPC]
                        return xt[kk - 2][:, 7 : 7 + RPC]
                    o, e = divmod(kk, 2)
                    return xt[e][:, 7 - o : 7 - o + RPC]

                for hh in range(HBLK):
                    ps = ph.tile([128, RPC], F32, tag="ps")
                    for kk in range(nk1):
                        nc.tensor.matmul(
                            ps[:],
                            w1_s[:, kk, hh * 128 : (hh + 1) * 128],
                            rhs_for(kk),
                            start=(kk == 0),
                            stop=(kk == nk1 - 1),
                        )
                    nc.scalar.activation(
                        ht[path][:, hh, :], ps[:], AF.Relu, bias=b1_s[:, hh : hh + 1]
                    )

        # ---------- W2 stream + stats + staging ----------
        with (
            tc.tile_pool(name="w2p", bufs=2) as w2p,
            tc.tile_pool(name="stp", bufs=1) as stp,
            tc.tile_pool(name="etp", bufs=2) as etp,
            tc.tile_pool(name="b2p", bufs=2) as b2p,
            tc.tile_pool(name="pw", bufs=4, space="PSUM") as pw,
        ):
            for path in range(2):
                w2dd = w2l_d if path == 0 else w2o_d
                for g in range(NG):
                    if use_b2:
                        b2s = b2p.tile([1, GW], BF16, tag="b2s")
                        nc.sync.dma_start(out=b2s[:], in_=b2_d[path, g])
                    stages = [
                        stp.tile([128, GW], BF16, tag=f"stg{rb}", bufs=2,
                                 name=f"stg{rb}")
                        for rb in range(NRB)
                    ]
                    for half in range(2):
                        w2t = [
                            w2p.tile([128, 2048], BF16, tag=f"w2t{hh}",
                                     name=f"w2t{hh}")
                            for hh in range(HBLK)
                        ]
                        for hh in range(HBLK):
                            nc.sync.dma_start(
                                out=w2t[hh][:],
                                in_=w2dd[g, hh, :, half * 2048 : (half + 1) * 2048],
                            )
                        for rb in range(NRB):
                            for vc in range(4):
                                col = half * 2048 + vc * 512
                                ps = pw.tile([128, 512], F32, tag="pw")
                                for hh in range(HBLK):
                                    nc.tensor.matmul(
                                        ps[:],
                                        ht[path][:, hh, rb * 128 : (rb + 1) * 128],
                                        w2t[hh][:, vc * 512 : (vc + 1) * 512],
                                        start=(hh == 0),
                                        stop=(hh == HBLK - 1) and not use_b2,
                                    )
                                if use_b2:
                                    nc.tensor.matmul(
                                        ps[:],
                                        ones_s[:],
                                        b2s[:, col : col + 512],
                                        start=False,
                                        stop=True,
                                    )
                                nc.vector.tensor_copy(
                                    stages[rb][:, col : col + 512], ps[:]
                                )
                    for rb in range(NRB):
                        stg = stages[rb]
                        nc.vector.tensor_copy(
                            acc[(path, rb, "sl")][:, g : g + 1],
                            stg[:, GV : GV + 1],
                        )
                        et = etp.tile([128, GV], BF16, tag="et", bufs=2)
                        nc.scalar.activation(
                            et[:], stg[:, :GV], AF.Exp,
                            accum_out=acc[(path, rb, "se")][:, g : g + 1],
                        )
                        dump = etp.tile([128, GV], BF16, tag="dump", bufs=1,
                                        name="dump")
                        nc.scalar.activation(
                            dump[:], stg[:, :GV], AF.Square,
                            accum_out=acc[(path, rb, "sq")][:, g : g + 1],
                        )
                        le = etp.tile([128, GV], BF16, tag="le", bufs=2)
                        nc.vector.tensor_mul(le[:], stg[:, :GV], et[:])
                        nc.scalar.activation(
                            dump[:], le[:], AF.Identity,
                            accum_out=acc[(path, rb, "sa")][:, g : g + 1],
                        )
                        nc.vector.tensor_reduce(
                            acc[(path, rb, "mx")][:, g : g + 1], stg[:, :GV],
                            axis=X_AXIS, op=ALU.max,
                        )
                        nc.sync.dma_start(out=ld[path, rb, g], in_=stg[:])

        # ---------- gate from local full-vocab stats ----------
        gate = sb.tile([128, NRB], F32, tag="gate")
        with tc.tile_pool(name="gt", bufs=1) as gt:
            red = {}
            for p in range(2):
                for nm, op in (("se", ALU.add), ("sa", ALU.add), ("sl", ALU.add),
                               ("sq", ALU.add), ("mx", ALU.max)):
                    t = gt.tile([128, NRB], F32, tag=f"r{p}{nm}", name=f"r{p}{nm}")
                    for rb in range(NRB):
                        nc.vector.tensor_reduce(
                            t[:, rb : rb + 1], acc[(p, rb, nm)][:],
                            axis=X_AXIS, op=op,
                        )
                    red[(p, nm)] = t

            feats = []  # [ent_lin, mx_lin, var_lin, ent_loc, mx_loc, var_loc]
            for p in range(2):
                S, A = red[(p, "se")], red[(p, "sa")]
                L, Q = red[(p, "sl")], red[(p, "sq")]
                rS = gt.tile([128, NRB], F32, tag=f"rS{p}")
                nc.vector.reciprocal(rS[:], S[:])
                AoS = gt.tile([128, NRB], F32, tag=f"AoS{p}")
                nc.vector.tensor_mul(AoS[:], A[:], rS[:])
                lnS = gt.tile([128, NRB], F32, tag=f"lnS{p}")
                nc.scalar.activation(lnS[:], S[:], AF.Ln)
                ent = gt.tile([128, NRB], F32, tag=f"ent{p}")
                nc.vector.tensor_sub(ent[:], lnS[:], AoS[:])
                mean = gt.tile([128, NRB], F32, tag=f"mean{p}")
                nc.vector.tensor_scalar_mul(mean[:], L[:], 1.0 / V)
                m2 = gt.tile([128, NRB], F32, tag=f"m2{p}")
                nc.vector.tensor_mul(m2[:], mean[:], mean[:])
                var = gt.tile([128, NRB], F32, tag=f"var{p}")
                nc.vector.tensor_scalar_mul(var[:], Q[:], 1.0 / V)
                nc.vector.tensor_sub(var[:], var[:], m2[:])
                feats += [ent, red[(p, "mx")], var]

            gacc = gt.tile([128, NRB], F32, tag="gacc")
            nc.vector.tensor_scalar_mul(gacc[:], feats[0][:], float(gate_w[0]))
            for i in range(1, 6):
                nc.vector.scalar_tensor_tensor(
                    out=gacc[:], in0=feats[i][:], scalar=float(gate_w[i]),
                    in1=gacc[:], op0=ALU.mult, op1=ALU.add,
                )
            nc.scalar.activation(
                gate[:], gacc[:], AF.Sigmoid, bias=float(gate_b), scale=1.0
            )

        # ---------- gated mix ----------
        with tc.tile_pool(name="mx", bufs=3) as mxp:
            for rb in range(NRB):
                for g in range(NG):
                    lin_s = mxp.tile([128, GV], BF16, tag="lin")
                    loc_s = mxp.tile([128, GV], BF16, tag="loc")
                    nc.sync.dma_start(out=lin_s[:], in_=ld[0, rb, g, :, 0:GV])
                    nc.sync.dma_start(out=loc_s[:], in_=ld[1, rb, g, :, 0:GV])
                    d = mxp.tile([128, GV], BF16, tag="d")
                    nc.vector.tensor_sub(d[:], lin_s[:], loc_s[:])
                    o = mxp.tile([128, GV], BF16, tag="o")
                    nc.vector.scalar_tensor_tensor(
                        out=o[:], in0=d[:], scalar=gate[:, rb : rb + 1],
                        in1=loc_s[:], op0=ALU.mult, op1=ALU.add,
                    )
                    nc.sync.dma_start(
                        out=out_d[rb, :, g * GV : (g + 1) * GV], in_=o[:]
                    )

    nc.compile()
    return nc


def _pack_w2(w2, b2):
    """[H, V] f32 -> [NG, HBLK, 128, GW] bf16 with col GV = group row-sum."""
    w2 = np.asarray(w2, np.float64)
    p = np.zeros((NG, HBLK, 128, GW), np.float32)
    for g in range(NG):
        blk = w2[:, g * GV : (g + 1) * GV]
        p[g, :, :, :GV] = blk.reshape(HBLK, 128, GV)
        p[g, :, :, GV] = blk.sum(axis=1).reshape(HBLK, 128)
    b2p = np.zeros((NG, 1, GW), np.float32)
    if b2 is not None:
        b2 = np.asarray(b2, np.float64)
        for g in range(NG):
            b2p[g, 0, :GV] = b2[g * GV : (g + 1) * GV]
            b2p[g, 0, GV] = b2[g * GV : (g + 1) * GV].sum()
    return _bf(p), b2p


def prepare_inputs(chars, emb, in_proj, decays, lin_W1, lin_b1, lin_W2, lin_b2,
                   loc_W1, loc_b1, loc_W2, loc_b2):
    chars = np.asarray(chars)
    x = np.asarray(emb, np.float32)[chars]            # [B, T, E]

    # exact decaying state bank on host (f64 recurrence)
    drive = x.astype(np.float64) @ np.asarray(in_proj, np.float64)  # [B,T,M]
    d = np.asarray(decays, np.float64)
    states = np.empty((B, T, M), np.float64)
    s = np.zeros((B, M), np.float64)
    for t in range(T):
        s = s * d + drive[:, t, :]
        states[:, t, :] = s

    # padded transposed x: [B, E, 7 + T]
    xt_all = np.zeros((B, E, 7 + T), np.float32)
    xt_all[:, :, 7:] = x.transpose(0, 2, 1)

    w2l, b2l = _pack_w2(lin_W2, lin_b2)
    w2o, b2o = _pack_w2(loc_W2, loc_b2)
    b2 = _bf(np.stack([b2l, b2o], axis=0))

    common = dict(
        w1l=_bf(np.asarray(lin_W1, np.float32).reshape(4, 128, H)),
        w1o=_bf(np.asarray(loc_W1, np.float32).reshape(16, 128, H)),
        b1l=np.ascontiguousarray(
            np.asarray(lin_b1, np.float32).reshape(HBLK, 128, 1)
        ),
        b1o=np.ascontiguousarray(
            np.asarray(loc_b1, np.float32).reshape(HBLK, 128, 1)
        ),
        w2l=w2l, w2o=w2o, b2=b2,
    )

    in_maps = []
    for c in range(N_CORES):
        b, t0 = divmod(c * RPC, T)
        xw = np.zeros((2, 128, XW), np.float32)
        xw[:, :, :263] = xt_all[b, :, t0 : t0 + 263].reshape(2, 128, 263)
        stw = np.ascontiguousarray(
            states[b, t0 : t0 + RPC, :].T.astype(np.float32).reshape(2, 128, RPC)
        )
        in_maps.append(dict(common, xt=_bf(xw), st=_bf(stw)))
    return in_maps


def assemble_output(results):
    full = np.empty((B * T, V), np.float32)
    for c in range(N_CORES):
        o = results[c]["out"].astype(np.float32)       # [NRB, 128, V]
        full[c * RPC : (c + 1) * RPC] = o.reshape(RPC, V)
    return np.ascontiguousarray(full.reshape(B, T, V))


_CACHE = {}


def _get_program(gate_W, gate_b, use_b2):
    gw = np.asarray(gate_W, np.float64).reshape(-1)
    gb = float(np.asarray(gate_b).reshape(-1)[0])
    key = (hash(gw.tobytes()), gb, use_b2)
    if key not in _CACHE:
        _CACHE[key] = build_program(gw, gb, use_b2)
    return _CACHE[key]


def kernel(chars, emb, in_proj, decays, lin_W1, lin_b1, lin_W2, lin_b2,
           loc_W1, loc_b1, loc_W2, loc_b2, gate_W, gate_b):
    use_b2 = bool(np.any(np.asarray(lin_b2)) or np.any(np.asarray(loc_b2)))
    nc = _get_program(gate_W, gate_b, use_b2)
    in_maps = prepare_inputs(chars, emb, in_proj, decays, lin_W1, lin_b1,
                             lin_W2, lin_b2, loc_W1, loc_b1, loc_W2, loc_b2)
    res = run_bass_kernel_spmd(nc, in_maps, CORE_IDS)
    return assemble_output(res.results)
